# revision 18
# baseline (speedup 1.0000x reference)
"""Modulated Conv2D (StyleGAN2-style) Trainium2 Bass kernel.

Problem shapes (hardcoded):
  x: [16, 256, 64, 64] f32    y: [16, 512] f32
  weights: [256, 256, 3, 3]   bias: [256]
  style_w: [256, 512]         style_b: [256]
  out: [16, 256, 64, 64] f32

Formulation: 1-D Winograd F(4,3) along H + direct 3-tap conv along W,
fp16 end-to-end (PE takes fp16 at bf16 rate; rel err ~1.5e-3), with the
per-sample style modulation folded into the weights:
  style[b,i] = y[b] @ style_w[i] + style_b[i]          (tiny PE matmul)
  U[kh,kx,i,o] = sum_ky G[kh,ky] * w[o,i,ky,kx]        (host, fp16)
  um[b][i,kh*3+kx,o] = U[...] * style[b,i]             (DVE)
  V transform per 16 tile-rows m (d_j = padded x rows 4m+j):
    h=d0-d2 a=d3-d1 b=d4-d2 c=d1+d2 e=d3+d4 f=d1-d2 g=d3-d4 k2=d5-d3
    V0=4h+b V1=e-4c V2=4f-g V3=2a+b V4=b-2a V5=k2-4a   (DVE TT+STT)
  M[kh][o,m,c] = sum_{i,kx} um[kh,kx,i,o] V[kh][i,m,c+kx]  (PE, f32 PSUM)
  md[kh] = M[kh]/wstd (+bias on kh=1)                  (scalar drain, fp16)
  p=md1+md2 q=md1-md2 r=md3+md4 s=md3-md4              (GPSIMD)
  out[4m+0]=md0+p+r  out[4m+1]=2s+q  out[4m+2]=4r+p  out[4m+3]=8s+q+md5
  wstd[b,o] = sqrt(sum_i W2[i,o]*style[b,i]^2 + eps)   (PE + Rsqrt)

Direct conv is 36 accumulating matmuls per 8 output rows; this needs 18
(2x fewer PE cycles). The transforms run on DVE/scalar/GPSIMD under the
PE stream; bias/demod are folded into the drain so no extra act pass.

Host-side prep (layout/dtype packing + static weight transform):
  - ut = G-transformed weights [Cin, 18, Cout] fp16 (lhsT-ready).
  - w2 = per-(i,o) weight square sums f32 (demod path operand).
  - swt/yT packed fp16 so style is a [512]-contraction PE matmul.
  - x zero-padded to [66,66] fp16, matmul-ready tiles.
  - device output fp16 with out-row (mod 4) planes separate; host
    interleaves + upcasts.

Sharding: data-parallel over batch, 2 samples per core across 8 cores.
"""

import numpy as np

import concourse.bass as bass
import concourse.tile as tile
from concourse import bacc, mybir
from concourse import bass_utils

EPS = 1e-8
P = 128
B_LOC = 2          # samples per core
B_FULL = 16
CIN, COUT = 256, 256
NI, NO = CIN // P, COUT // P   # 2, 2
S = 512
NS = S // P        # 4 style contraction blocks
KH, KX = 6, 3      # winograd points along H, direct taps along W
KK = KH * KX       # 18 lhsT planes
KA = 9             # first kk-chunk of the ut loads
H = W = 64
HP, WP = H + 2, W + 2  # zero-padded image
TR = H // 4        # 16 winograd tile-rows
N_CORES = 8
ROWS_A = 34        # rows in the first half of each x tile load

F32 = mybir.dt.float32
F16 = mybir.dt.float16
AF = mybir.ActivationFunctionType
OP = mybir.AluOpType


def _chain(instrs, reason):
    """Force program order on one engine (guides the tile scheduler)."""
    for a, b in zip(instrs[1:], instrs[:-1]):
        bass._add_dep_helper(a.ins, b.ins, sync=False, reason=reason)


def build_conv2dmod(nc):
    xp = nc.dram_tensor("xp", [B_LOC, CIN, HP, WP], F16, kind="ExternalInput")
    ut = nc.dram_tensor("ut", [CIN, KK, COUT], F16, kind="ExternalInput")
    swt = nc.dram_tensor("swt", [P, NS, CIN + B_LOC], F16, kind="ExternalInput")
    # host-prepacked [128, 5] f32: bias cols (2), style_b cols (2), eps (1)
    cst = nc.dram_tensor("cst", [P, 2 * NO + 1], F32, kind="ExternalInput")
    # host-prepacked W2T [i_part, it, o] f32
    w2 = nc.dram_tensor("w2", [P, NI, COUT], F32, kind="ExternalInput")
    # out-row (mod 4) planes separate; host interleaves
    out = nc.dram_tensor("out", [B_LOC, COUT, 4, TR, W], F16,
                         kind="ExternalOutput")

    with tile.TileContext(nc) as tc:
        with (
            tc.tile_pool(name="consts", bufs=1) as consts,
            tc.tile_pool(name="sh_pool", bufs=1) as sh_pool,
            tc.tile_pool(name="md_pool", bufs=3) as md_pool,
            tc.tile_pool(name="um_pool", bufs=1) as um_pool,
            tc.tile_pool(name="xs_pool", bufs=1) as xs_pool,
            tc.tile_pool(name="v_pool", bufs=1) as v_pool,
            tc.tile_pool(name="out_pool", bufs=3) as out_pool,
            tc.tile_pool(name="psum", bufs=1, space="PSUM") as psum,
        ):
            # ---------------- tiles ----------------
            swt_t = consts.tile([P, NS, CIN + B_LOC], F16)
            ut_t = [consts.tile([P, KK, COUT], F16, name=f"ut{i}", tag=f"ut{i}")
                    for i in range(NI)]
            w2_t = consts.tile([P, NI, COUT], F32)
            cst_t = consts.tile([P, 2 * NO + 1], F32)
            xs = {}
            vt = {}
            for s in range(B_LOC):
                for cb in range(NI):
                    xs[(s, cb)] = xs_pool.tile(
                        [P, HP, WP], F16, name=f"xs{s}_{cb}", tag=f"xs{s}_{cb}")
                    vt[(s, cb)] = v_pool.tile(
                        [P, KH, TR, WP], F16, name=f"vt{s}_{cb}",
                        tag=f"vt{s}_{cb}")
            um = {}
            for s in range(B_LOC):
                for it in range(NI):
                    um[(s, it)] = um_pool.tile(
                        [P, KK, COUT], F16, name=f"um{s}_{it}",
                        tag=f"um{s}_{it}")

            def bias_ap(ot):
                return cst_t[:, ot:ot + 1]

            def style_b_ap(it):
                return cst_t[:, NO + it:NO + it + 1]

            eps_ap = cst_t[:, 2 * NO:2 * NO + 1]

            # pre-warm the ACT table that Rsqrt lives in; dependency-free so
            # the async table load fires before anything else on scalar
            warm_src = consts.tile([P, 1], F32)
            nc.gpsimd.memset(warm_src[:], EPS)
            lafs_warm = consts.tile([P, 1], F32)
            warm_i = nc.scalar.activation(lafs_warm[:], warm_src[:], AF.Sqrt)

            # ------------- DMA rings, ordered by when they gate compute -------
            # the two HWDGE rings share ~330 GB/s; order both by gate time
            scalar_ring = [
                nc.scalar.dma_start(swt_t[:], swt.ap()),
                nc.scalar.dma_start(w2_t[:], w2.ap()),
                nc.scalar.dma_start(ut_t[0][:, 0:KA, :], ut.ap()[0:P, 0:KA]),
                nc.scalar.dma_start(ut_t[0][:, KA:KK, :], ut.ap()[0:P, KA:KK]),
            ]
            _chain([warm_i] + scalar_ring, "warm then scalar ring order")

            def load_x(s, cb, half):
                r = slice(0, ROWS_A) if half == 0 else slice(ROWS_A, HP)
                nc.sync.dma_start(xs[(s, cb)][:, r, :],
                                  xp.ap()[s, cb * P:(cb + 1) * P, r, :])

            nc.sync.dma_start(cst_t[:], cst.ap())
            load_x(0, 0, 0)
            nc.sync.dma_start(ut_t[1][:, 0:KA, :], ut.ap()[P:2 * P, 0:KA])
            load_x(0, 1, 0)
            nc.sync.dma_start(ut_t[1][:, KA:KK, :], ut.ap()[P:2 * P, KA:KK])
            load_x(0, 0, 1)
            load_x(0, 1, 1)

            # ---------- style (PE): [P(cin), B_LOC] per cin block ----------
            style_col = []
            style2 = []
            for it in range(NI):
                ps = psum.tile([P, B_LOC], F32, name=f"styp{it}", tag=f"b{6 + it}")
                for sb in range(NS):
                    nc.tensor.matmul(
                        ps[:], swt_t[:, sb, it * P:(it + 1) * P],
                        swt_t[:, sb, CIN:CIN + B_LOC],
                        start=(sb == 0), stop=(sb == NS - 1),
                    )
                sc = consts.tile([P, B_LOC], F32, name=f"stc{it}", tag=f"stc{it}")
                nc.scalar.activation(sc[:], ps[:], AF.Identity,
                                     bias=style_b_ap(it))
                style_col.append(sc)

            # ---------- per-sample modulated U weights (DVE) ----------
            def make_um(s, it):
                t = um[(s, it)]
                for k0, k1 in ((0, KA), (KA, KK)):
                    nc.vector.tensor_scalar_mul(
                        t[:, k0:k1, :], ut_t[it][:, k0:k1, :],
                        style_col[it][:, s:s + 1])

            # ---------- V input transform (DVE) ----------
            def make_v(s, cb, m0, n):
                x_t = xs[(s, cb)]
                v = vt[(s, cb)]

                def d(k):
                    a = 4 * m0 + k
                    return x_t[:, a:a + 4 * (n - 1) + 1:4, :]

                def sh(j):
                    t = sh_pool.tile([P, TR, WP], F16, name=f"sh{j}",
                                     tag=f"sh{j}")
                    return t[:, 0:n, :]

                ms = slice(m0, m0 + n)
                h = sh(0); nc.vector.tensor_sub(h, d(0), d(2))
                a_ = sh(1); nc.vector.tensor_sub(a_, d(3), d(1))
                b_ = sh(2); nc.vector.tensor_sub(b_, d(4), d(2))
                c_ = sh(3); nc.vector.tensor_add(c_, d(1), d(2))
                e_ = sh(4); nc.vector.tensor_add(e_, d(3), d(4))
                f_ = sh(5); nc.vector.tensor_sub(f_, d(1), d(2))
                g_ = sh(6); nc.vector.tensor_sub(g_, d(3), d(4))
                k_ = sh(7); nc.vector.tensor_sub(k_, d(5), d(3))
                stt = nc.vector.scalar_tensor_tensor
                stt(v[:, 0, ms, :], h, 4.0, b_, OP.mult, OP.add)
                stt(v[:, 1, ms, :], c_, -4.0, e_, OP.mult, OP.add)
                stt(v[:, 2, ms, :], f_, 4.0, g_, OP.mult, OP.subtract)
                stt(v[:, 3, ms, :], a_, 2.0, b_, OP.mult, OP.add)
                stt(v[:, 4, ms, :], a_, -2.0, b_, OP.mult, OP.add)
                stt(v[:, 5, ms, :], a_, -4.0, k_, OP.mult, OP.add)

            # DVE program order by data readiness (see module docstring)
            make_v(0, 0, 0, 8)
            style2 = []
            for it in range(NI):
                s2 = consts.tile([P, B_LOC], F32, name=f"st2{it}", tag=f"st2{it}")
                nc.vector.tensor_mul(s2[:], style_col[it][:], style_col[it][:])
                style2.append(s2)
            make_um(0, 0)
            make_v(0, 1, 0, 8)
            make_um(0, 1)

            # ---------- demod path ----------
            # sigma[o_part, b] = sum_i W2T[i,o] * style2[i,b]  (PE, f32);
            # the DVE reciprocal slots in right after um(0,1) so winv is
            # ready before the first chunk's drains
            winv = []
            for ot in range(NO):
                ps = psum.tile([P, B_LOC], F32, name=f"sig{ot}", tag=f"b{6 + ot}")
                for it in range(NI):
                    nc.tensor.matmul(
                        ps[:], w2_t[:, it, ot * P:(ot + 1) * P], style2[it][:],
                        start=(it == 0), stop=(it == NI - 1),
                    )
                wstd = consts.tile([P, B_LOC], F32, name=f"wstd{ot}",
                                   tag=f"wstd{ot}")
                nc.scalar.activation(wstd[:], ps[:], AF.Sqrt, bias=eps_ap)
                wi = consts.tile([P, B_LOC], F32, name=f"winv{ot}",
                                 tag=f"winv{ot}")
                nc.vector.reciprocal(wi[:], wstd[:])
                winv.append(wi)

            make_v(0, 0, 8, 8)
            make_v(0, 1, 8, 8)

            # ---------- main conv: 36 matmuls per 8-tile-row chunk ----------
            ci = [0]  # global conv-chunk counter for PSUM bank cycling

            def alloc_pcs(s, ot, tr0):
                base = 6 * ci[0]
                ci[0] += 1
                return [psum.tile([P, 8, W], F32, name=f"pc{s}{ot}{tr0}_{k}",
                                  tag=f"b{(base + k) % 8}")
                        for k in range(KH)]

            def mm_half(s, ot, tr0, trsz, it, pcs):
                for kh in range(KH):
                    for kx in range(KX):
                        lhsT = um[(s, it)][:, kh * KX + kx,
                                           ot * P:(ot + 1) * P]
                        rhs = vt[(s, it)][:, kh, tr0:tr0 + trsz, kx:kx + W]
                        nc.tensor.matmul(
                            pcs[kh][:, 0:trsz, :], lhsT, rhs,
                            start=(it == 0 and kx == 0),
                            stop=(it == NI - 1 and kx == KX - 1),
                        )

            def mm_block(s, ot, tr0, trsz):
                pcs = alloc_pcs(s, ot, tr0)
                for it in range(NI):
                    mm_half(s, ot, tr0, trsz, it, pcs)
                return pcs

            def out_block(s, ot, tr0, trsz, pcs):
                z = slice(0, trsz)
                nm = f"{s}{ot}{tr0}"
                # drains: demod (and bias, on kh=1) folded in, fp16 out
                md = []
                for j in range(KH):
                    t = md_pool.tile([P, 8, W], F16, name=f"md{nm}_{j}",
                                     tag=f"md{j}")
                    kw = dict(bias=bias_ap(ot)) if j == 1 else {}
                    nc.scalar.activation(t[:, z, :], pcs[j][:, z, :],
                                         AF.Identity,
                                         scale=winv[ot][:, s:s + 1], **kw)
                    md.append(t)
                pp = md_pool.tile([P, 8, W], F16, name=f"pp{nm}", tag="pp")
                qq = md_pool.tile([P, 8, W], F16, name=f"qq{nm}", tag="qq")
                rr = md_pool.tile([P, 8, W], F16, name=f"rr{nm}", tag="rr")
                ss = md_pool.tile([P, 8, W], F16, name=f"ss{nm}", tag="ss")
                nc.gpsimd.tensor_add(pp[:, z, :], md[1][:, z, :], md[2][:, z, :])
                nc.gpsimd.tensor_sub(qq[:, z, :], md[1][:, z, :], md[2][:, z, :])
                nc.gpsimd.tensor_add(rr[:, z, :], md[3][:, z, :], md[4][:, z, :])
                nc.gpsimd.tensor_sub(ss[:, z, :], md[3][:, z, :], md[4][:, z, :])
                oh = out_pool.tile([P, 4, 8, W], F16, name=f"oh{nm}", tag="oh")
                t0 = md_pool.tile([P, 8, W], F16, name=f"t0{nm}", tag="t0")
                t3 = md_pool.tile([P, 8, W], F16, name=f"t3{nm}", tag="t3")
                stt = nc.vector.scalar_tensor_tensor
                nc.vector.tensor_add(t0[:, z, :], md[0][:, z, :], pp[:, z, :])
                nc.vector.tensor_add(oh[:, 0, z, :], t0[:, z, :], rr[:, z, :])
                stt(oh[:, 1, z, :], ss[:, z, :], 2.0, qq[:, z, :],
                    OP.mult, OP.add)
                stt(oh[:, 2, z, :], rr[:, z, :], 4.0, pp[:, z, :],
                    OP.mult, OP.add)
                stt(t3[:, z, :], ss[:, z, :], 8.0, qq[:, z, :],
                    OP.mult, OP.add)
                nc.vector.tensor_add(oh[:, 3, z, :], t3[:, z, :], md[5][:, z, :])
                nc.sync.dma_start(
                    out.ap()[s, ot * P:(ot + 1) * P, :, tr0:tr0 + trsz, :],
                    oh[:, :, z, :])

            # head chunk split by cin-block: it=0 needs only ut0 + x(cb0) h0
            pcs_c0 = alloc_pcs(0, 0, 0)
            mm_half(0, 0, 0, 8, 0, pcs_c0)
            mm_half(0, 0, 0, 8, 1, pcs_c0)
            out_block(0, 0, 0, 8, pcs_c0)
            # sample-1 x loads queue behind the first out store
            for half in range(2):
                for cb in range(NI):
                    load_x(1, cb, half)
            # sample-1 weight mod + V transform interleaved in small blocks
            # between chunks so chunk combines never queue behind 17us of
            # DVE transform work (that backlog stalls drains -> PSUM -> PE)
            make_um(1, 0)
            make_um(1, 1)
            # (0,1,tr0) before (0,0,tr8): its V block is ready earlier
            out_block(0, 1, 0, 8, mm_block(0, 1, 0, 8))
            make_v(1, 0, 0, 8)
            out_block(0, 0, 8, 8, mm_block(0, 0, 8, 8))
            make_v(1, 0, 8, 8)
            out_block(0, 1, 8, 8, mm_block(0, 1, 8, 8))
            make_v(1, 1, 0, 8)
            out_block(1, 0, 0, 8, mm_block(1, 0, 0, 8))
            make_v(1, 1, 8, 8)
            out_block(1, 0, 8, 8, mm_block(1, 0, 8, 8))
            # shrinking tail so the final drain after the last matmul is short
            for tr0, trsz in ((0, 8), (8, 4), (12, 2), (14, 2)):
                out_block(1, 1, tr0, trsz, mm_block(1, 1, tr0, trsz))
    return nc


_CACHED_NC = None


def _get_nc():
    global _CACHED_NC
    if _CACHED_NC is None:
        nc = bacc.Bacc("TRN2", target_bir_lowering=False, debug=False,
                       num_devices=N_CORES)
        build_conv2dmod(nc)
        nc.compile()
        _CACHED_NC = nc
    return _CACHED_NC


def kernel(x, y, weights, bias, style_w, style_b, _trace=False):
    x = np.asarray(x, dtype=np.float32)
    y = np.asarray(y, dtype=np.float32)
    weights = np.asarray(weights, dtype=np.float32)
    bias = np.asarray(bias, dtype=np.float32)
    style_w = np.asarray(style_w, dtype=np.float32)
    style_b = np.asarray(style_b, dtype=np.float32)

    # host-side layout packing (see module docstring)
    G = np.array([[1 / 4, 0, 0],
                  [-1 / 6, -1 / 6, -1 / 6],
                  [-1 / 6, 1 / 6, -1 / 6],
                  [1 / 24, 1 / 12, 1 / 6],
                  [1 / 24, -1 / 12, 1 / 6],
                  [0, 0, 1]], np.float64)
    # U[i, kh, kx, o] = sum_ky G[kh,ky] w[o,i,ky,kx] -> [Cin, 18, Cout] fp16
    U = np.einsum("hk,oikx->ihxo", G, weights.astype(np.float64))
    ut = np.ascontiguousarray(U.reshape(CIN, KK, COUT)).astype(np.float16)
    # W2[i,o] = sum_kk w[o,i,kk]^2 -> [P, NI, COUT] f32
    W2 = np.einsum("oikl->io", weights.astype(np.float64) ** 2).astype(np.float32)
    w2 = np.ascontiguousarray(W2.reshape(NI, P, COUT).transpose(1, 0, 2))
    swtf = style_w.T.reshape(NS, P, CIN).transpose(1, 0, 2)  # [sp, sb, i]
    ytf = y.T.reshape(NS, P, B_FULL).transpose(1, 0, 2)      # [sp, sb, b_full]
    xp = np.zeros((B_FULL, CIN, HP, WP), dtype=np.float16)
    xp[:, :, 1:H + 1, 1:W + 1] = x.astype(np.float16)
    cst = np.empty((P, 2 * NO + 1), dtype=np.float32)
    cst[:, 0:NO] = bias.reshape(NO, P).T
    cst[:, NO:2 * NO] = style_b.reshape(NI, P).T
    cst[:, 2 * NO] = EPS

    nc = _get_nc()
    in_maps = []
    for c in range(N_CORES):
        swt_c = np.empty((P, NS, CIN + B_LOC), dtype=np.float16)
        swt_c[:, :, :CIN] = swtf
        swt_c[:, :, CIN:] = ytf[:, :, c * B_LOC:(c + 1) * B_LOC]
        in_maps.append({
            "xp": np.ascontiguousarray(xp[c * B_LOC:(c + 1) * B_LOC]),
            "ut": ut,
            "swt": swt_c,
            "cst": cst,
            "w2": w2,
        })
    res = bass_utils.run_bass_kernel_spmd(
        nc, in_maps, core_ids=list(range(N_CORES)), trace=_trace
    )
    # out planes: [B_LOC, COUT, 4, 16, 64] -> interleave rows mod 4
    out = np.concatenate(
        [r["out"].transpose(0, 1, 3, 2, 4).reshape(B_LOC, COUT, H, W)
         for r in res.results], axis=0).astype(np.float32)
    if _trace:
        kernel.last_results = res
    return out
